# revision 7
# baseline (speedup 1.0000x reference)
"""Trainium2 Bass kernel: contrastive hinge loss over an 8192x8192 pairwise
mean-squared-distance matrix, sharded over 8 NeuronCores (1024 z1 rows each).

Math (reference):
  dist[m,n] = (sq1[m] + sq2[n] - 2*z1[m]@z2[n]) / D
  lossP = sum(relu(diag(dist) - MARGIN_POS)) / B
  lossN = sum(relu(MARGIN_NEG - dist) * (1-eye)) / (B*(B-1))
  out   = 1.5*lossP + 0.5*lossN

Device-side identity used for the off-diag hinge: with the PE computing
  acc[m,n] = z1[m]@z2[n] - 0.5*sq2[n]          (4 matmuls + one rank-1 fold)
we have
  relu(acc + (0.2*D - 0.5*sq1[m])) = (D/2) * relu(MARGIN_NEG - dist[m,n])
so one ScalarE activation (per-partition bias + free-axis accumulate) reduces
each PSUM tile to per-row hinge sums.  Diagonal terms (needed exactly for
lossP, and subtracted from the off-diag sum) come from a separate cheap fp32
elementwise path over the matching z1/z2 row shards.
"""

import numpy as np

B = 8192
D = 512
NCORES = 8
M = B // NCORES          # 1024 z1 rows per core
P = 128
MT = M // P              # 8 row tiles per core
NCHUNK = 512             # z2 column chunk (one PSUM bank wide)
NPAIRS = B // (2 * NCHUNK)   # 8 chunk pairs
KT = D // P              # 4 contraction subtiles
MARGIN_NEG = 0.4
MARGIN_POS = 0.01
MAIN_COLS = NPAIRS * MT          # 64 hinge-accum columns
STATS_COLS = MAIN_COLS + 2 * MT  # + lossP cols + diag-correction cols

_NC_CACHE = {}


def _legalize_waits(nc, max_waits=1):
    """This toolchain's walrus accepts only ONE sync wait command per
    instruction ("Too many sync wait commands" otherwise), while Tile
    attaches all required waits to the consuming instruction.  Hoist every
    wait beyond the first onto standalone same-engine NoOps placed directly
    before the consumer — per-engine program order keeps semantics
    identical."""
    from concourse import mybir

    for f in nc.m.functions:
        for blk in f.blocks:
            out = []
            changed = False
            for inst in blk.instructions:
                si = getattr(inst, "sync_info", None)
                waits = list(si.on_wait) if si is not None else []
                if len(waits) > max_waits:
                    for i, w in enumerate(waits[max_waits:]):
                        out.append(mybir.InstNoOp(
                            name=f"{inst.name}_hw{i}",
                            sync_info=mybir.SyncInfo(on_wait=[w], on_update=[]),
                            engine=inst.engine,
                            bass_nofuse=True,
                        ))
                    inst.sync_info = mybir.SyncInfo(
                        on_wait=waits[:max_waits], on_update=list(si.on_update))
                    changed = True
                out.append(inst)
            if changed:
                blk.instructions = out


def build_nc():
    import concourse.bass as bass
    import concourse.tile as tile
    from concourse import mybir

    f32 = mybir.dt.float32
    Relu = mybir.ActivationFunctionType.Relu
    Square = mybir.ActivationFunctionType.Square
    Copy = mybir.ActivationFunctionType.Copy

    nc = bass.Bass()
    z1tT = nc.dram_tensor("z1tT", [D, M], f32, kind="ExternalInput")
    z2T = nc.dram_tensor("z2T", [D, B], f32, kind="ExternalInput")
    z1s = nc.dram_tensor("z1s", [M, D], f32, kind="ExternalInput")
    z2s = nc.dram_tensor("z2s", [M, D], f32, kind="ExternalInput")
    stats = nc.dram_tensor("stats", [P, STATS_COLS], f32, kind="ExternalOutput")

    z1tT_r = z1tT.rearrange("(kt p) m -> p kt m", p=P)
    z2T_r = z2T.rearrange("(kt p) n -> p kt n", p=P)

    with tile.TileContext(nc) as tc:
        with (
            tc.tile_pool(name="singles", bufs=1) as singles,
            tc.tile_pool(name="z2pool", bufs=3) as z2pool,
            tc.tile_pool(name="sqpool", bufs=4) as sqpool,
            tc.tile_pool(name="rowpool", bufs=3) as rowpool,
            tc.tile_pool(name="diagpool", bufs=3) as diagpool,
            tc.tile_pool(name="scratch", bufs=3) as scratch,
            tc.tile_pool(name="psum_row", bufs=2, space="PSUM") as psum_row,
            tc.tile_pool(name="psum_main", bufs=3, space="PSUM") as psum_main,
        ):
            # ---- resident tiles ----
            z1tT_sb = singles.tile([P, KT, M], f32)
            nc.sync.dma_start(out=z1tT_sb, in_=z1tT_r)
            ones_k = singles.tile([P, 1], f32)      # lhsT for partition-sums
            nc.vector.memset(ones_k, 1.0)
            ones_m = singles.tile([1, P], f32)      # lhsT for rank-1 row fold
            nc.vector.memset(ones_m, 1.0)
            stats_sb = singles.tile([P, STATS_COLS], f32)
            sdsum = singles.tile([P, MT], f32)      # sum_d (z1-z2)^2 per row
            sq1sum = singles.tile([P, MT], f32)     # sum_d z1^2 per row
            biasP = singles.tile([P, MT], f32)      # 0.2*D - 0.5*sq1
            bias_mpos = singles.tile([P, 1], f32)
            nc.vector.memset(bias_mpos, -MARGIN_POS)
            bias_mneg = singles.tile([P, 1], f32)
            nc.vector.memset(bias_mneg, MARGIN_NEG)

            # ---- diagonal path (exact, fp32) ----
            for mt in range(MT):
                t1 = diagpool.tile([P, D], f32, tag="t1")
                nc.sync.dma_start(out=t1, in_=z1s[mt * P:(mt + 1) * P, :])
                t2 = diagpool.tile([P, D], f32, tag="t2")
                nc.sync.dma_start(out=t2, in_=z2s[mt * P:(mt + 1) * P, :])
                dsub = diagpool.tile([P, D], f32, tag="dsub")
                # copy-then-subtract: a DVE TensorTensor has too few sync
                # wait slots to wait on two DMA queue semaphores at once
                nc.vector.tensor_copy(out=dsub, in_=t2)
                nc.vector.tensor_tensor(dsub, t1, dsub, mybir.AluOpType.subtract)
                tr1 = diagpool.tile([P, D], f32, tag="tr1")
                nc.scalar.activation(tr1, dsub, Square,
                                     accum_out=sdsum[:, mt:mt + 1])
                tr2 = diagpool.tile([P, D], f32, tag="tr2")
                nc.scalar.activation(tr2, t1, Square,
                                     accum_out=sq1sum[:, mt:mt + 1])

            nc.scalar.activation(biasP, sq1sum, Copy,
                                 bias=0.5 * MARGIN_NEG * D, scale=-0.5)
            nc.scalar.activation(stats_sb[:, MAIN_COLS:MAIN_COLS + MT], sdsum,
                                 Relu, bias=bias_mpos, scale=1.0 / D)
            nc.scalar.activation(stats_sb[:, MAIN_COLS + MT:STATS_COLS], sdsum,
                                 Relu, bias=bias_mneg, scale=-1.0 / D)

            # ---- main loop over z2 column-chunk pairs ----
            for pr in range(NPAIRS):
                pair = []
                for half in range(2):
                    ci = pr * 2 + half
                    z2c = z2pool.tile([P, KT, NCHUNK], f32, tag="z2c")
                    nc.sync.dma_start(
                        out=z2c,
                        in_=z2T_r[:, :, ci * NCHUNK:(ci + 1) * NCHUNK])
                    # sq2 chunk row: ones^T @ (z2c*z2c), accumulated over KT
                    prow = psum_row.tile([1, NCHUNK], f32, tag="prow")
                    for k in range(KT):
                        z2sq = sqpool.tile([P, NCHUNK], f32, tag="z2sq")
                        nc.vector.tensor_mul(z2sq, z2c[:, k, :], z2c[:, k, :])
                        nc.tensor.matmul(prow, lhsT=ones_k, rhs=z2sq,
                                         start=(k == 0), stop=(k == KT - 1))
                    msq2h = rowpool.tile([1, NCHUNK], f32, tag="msq2h")
                    nc.scalar.activation(msq2h, prow, Copy, scale=-0.5)
                    pair.append((z2c, msq2h))

                for mt in range(MT):
                    pbig = psum_main.tile([P, 2, NCHUNK], f32, tag="pbig")
                    for half in range(2):
                        z2c, msq2h = pair[half]
                        pm = pbig[:, half, :]
                        for k in range(KT):
                            nc.tensor.matmul(
                                pm,
                                lhsT=z1tT_sb[:, k, mt * P:(mt + 1) * P],
                                rhs=z2c[:, k, :],
                                start=(k == 0), stop=False)
                        nc.tensor.matmul(pm, lhsT=ones_m, rhs=msq2h,
                                         start=False, stop=True)
                    hout = scratch.tile([P, 2, NCHUNK], f32, tag="hout")
                    col = pr * MT + mt
                    nc.scalar.activation(
                        hout, pbig, Relu, bias=biasP[:, mt:mt + 1], scale=1.0,
                        accum_out=stats_sb[:, col:col + 1])

            nc.sync.dma_start(out=stats[:, :], in_=stats_sb)
    return nc


def get_nc():
    if "nc" not in _NC_CACHE:
        nc = build_nc()
        nc.finalize()
        _legalize_waits(nc)
        _NC_CACHE["nc"] = nc
    return _NC_CACHE["nc"]


def prep_in_maps(z1, z2):
    z1 = np.ascontiguousarray(np.asarray(z1, dtype=np.float32))
    z2 = np.ascontiguousarray(np.asarray(z2, dtype=np.float32))
    z2T = np.ascontiguousarray(z2.T)
    in_maps = []
    for i in range(NCORES):
        sl = slice(i * M, (i + 1) * M)
        in_maps.append({
            "z1tT": np.ascontiguousarray(z1[sl].T),
            "z2T": z2T,
            "z1s": np.ascontiguousarray(z1[sl]),
            "z2s": np.ascontiguousarray(z2[sl]),
        })
    return in_maps


def gather(results):
    """Combine per-core stats tensors into the final scalar loss."""
    mainsum = 0.0
    lossP_sum = 0.0
    h0sum = 0.0
    for r in results:
        s = np.asarray(r["stats"], dtype=np.float64)
        mainsum += s[:, :MAIN_COLS].sum()
        lossP_sum += s[:, MAIN_COLS:MAIN_COLS + MT].sum()
        h0sum += s[:, MAIN_COLS + MT:STATS_COLS].sum()
    lossN = (mainsum * (2.0 / D) - h0sum) / (B * (B - 1))
    lossP = lossP_sum / B
    return np.array(1.5 * lossP + 0.5 * lossN, dtype=np.float32)


def run(z1, z2, trace=False, trace_cores=None):
    from concourse.bass_utils import run_bass_kernel_spmd

    nc = get_nc()
    in_maps = prep_in_maps(z1, z2)
    kwargs = {}
    if trace:
        kwargs["trace"] = True
        if trace_cores is not None:
            kwargs["trace_cores"] = trace_cores
    res = run_bass_kernel_spmd(nc, in_maps, core_ids=list(range(NCORES)),
                               **kwargs)
    return gather(res.results), res


def kernel(z1, z2):
    val, _ = run(z1, z2)
    return val


# revision 9
# speedup vs baseline: 1.4530x; 1.4530x over previous
"""Trainium2 Bass kernel: contrastive hinge loss over an 8192x8192 pairwise
mean-squared-distance matrix, sharded over 8 NeuronCores (1024 z1 rows each).

Math (reference):
  dist[m,n] = (sq1[m] + sq2[n] - 2*z1[m]@z2[n]) / D
  lossP = sum(relu(diag(dist) - MARGIN_POS)) / B
  lossN = sum(relu(MARGIN_NEG - dist) * (1-eye)) / (B*(B-1))
  out   = 1.5*lossP + 0.5*lossN

Device-side identity used for the off-diag hinge: with the PE computing
  acc[m,n] = z1[m]@z2[n] - 0.5*sq2[n]          (4 matmuls + one rank-1 fold)
we have
  relu(acc + (0.2*D - 0.5*sq1[m])) = (D/2) * relu(MARGIN_NEG - dist[m,n])
so one ScalarE activation (per-partition bias + free-axis accumulate) reduces
each PSUM tile to per-row hinge sums.  Diagonal terms (needed exactly for
lossP, and subtracted from the off-diag sum) come from a separate cheap fp32
elementwise path over the matching z1/z2 row shards.
"""

import numpy as np

B = 8192
D = 512
NCORES = 8
M = B // NCORES          # 1024 z1 rows per core
P = 128
MT = M // P              # 8 row tiles per core
NCHUNK = 512             # z2 column chunk (one PSUM bank wide)
NPAIRS = B // (2 * NCHUNK)   # 8 chunk pairs
KT = D // P              # 4 contraction subtiles
MARGIN_NEG = 0.4
MARGIN_POS = 0.01
MAIN_COLS = NPAIRS * MT          # 64 hinge-accum columns
STATS_COLS = MAIN_COLS + 2 * MT  # + lossP cols + diag-correction cols

_NC_CACHE = {}


def _legalize_waits(nc, max_waits=1):
    """This toolchain's walrus accepts only ONE sync wait command per
    instruction ("Too many sync wait commands" otherwise), while Tile
    attaches all required waits to the consuming instruction.  Hoist every
    wait beyond the first onto standalone same-engine NoOps placed directly
    before the consumer — per-engine program order keeps semantics
    identical."""
    from concourse import mybir

    for f in nc.m.functions:
        for blk in f.blocks:
            out = []
            changed = False
            for inst in blk.instructions:
                si = getattr(inst, "sync_info", None)
                waits = list(si.on_wait) if si is not None else []
                if len(waits) > max_waits:
                    for i, w in enumerate(waits[max_waits:]):
                        out.append(mybir.InstNoOp(
                            name=f"{inst.name}_hw{i}",
                            sync_info=mybir.SyncInfo(on_wait=[w], on_update=[]),
                            engine=inst.engine,
                            bass_nofuse=True,
                        ))
                    inst.sync_info = mybir.SyncInfo(
                        on_wait=waits[:max_waits], on_update=list(si.on_update))
                    changed = True
                out.append(inst)
            if changed:
                blk.instructions = out


def build_nc():
    import concourse.bass as bass
    import concourse.tile as tile
    from concourse import mybir

    f32 = mybir.dt.float32
    bf16 = mybir.dt.bfloat16
    Relu = mybir.ActivationFunctionType.Relu
    Square = mybir.ActivationFunctionType.Square
    Copy = mybir.ActivationFunctionType.Copy

    nc = bass.Bass()
    z1tT = nc.dram_tensor("z1tT", [D, M], bf16, kind="ExternalInput")
    z2T = nc.dram_tensor("z2T", [D, B], bf16, kind="ExternalInput")
    z1s = nc.dram_tensor("z1s", [M, D], f32, kind="ExternalInput")
    z2s = nc.dram_tensor("z2s", [M, D], f32, kind="ExternalInput")
    stats = nc.dram_tensor("stats", [P, STATS_COLS], f32, kind="ExternalOutput")

    z1tT_r = z1tT.rearrange("(kt p) m -> p kt m", p=P)
    z2T_r = z2T.rearrange("(kt p) n -> p kt n", p=P)

    with tile.TileContext(nc) as tc:
        with (
            tc.tile_pool(name="singles", bufs=1) as singles,
            tc.tile_pool(name="z2pool", bufs=3) as z2pool,
            tc.tile_pool(name="sqpool", bufs=4) as sqpool,
            tc.tile_pool(name="rowpool", bufs=3) as rowpool,
            tc.tile_pool(name="diagpool", bufs=3) as diagpool,
            tc.tile_pool(name="scratch", bufs=3) as scratch,
            tc.tile_pool(name="psum_row", bufs=2, space="PSUM") as psum_row,
            tc.tile_pool(name="psum_main", bufs=3, space="PSUM") as psum_main,
        ):
            # ---- resident tiles ----
            z1tT_sb = singles.tile([P, KT, M], bf16)
            nc.sync.dma_start(out=z1tT_sb, in_=z1tT_r)
            ones_k = singles.tile([P, 1], bf16)     # lhsT for partition-sums
            nc.vector.memset(ones_k, 1.0)
            ones_m = singles.tile([1, P], f32)      # lhsT for rank-1 row fold
            nc.vector.memset(ones_m, 1.0)
            stats_sb = singles.tile([P, STATS_COLS], f32)
            sdsum = singles.tile([P, MT], f32)      # sum_d (z1-z2)^2 per row
            sq1sum = singles.tile([P, MT], f32)     # sum_d z1^2 per row
            biasP = singles.tile([P, MT], f32)      # 0.2*D - 0.5*sq1
            bias_mpos = singles.tile([P, 1], f32)
            nc.vector.memset(bias_mpos, -MARGIN_POS)
            bias_mneg = singles.tile([P, 1], f32)
            nc.vector.memset(bias_mneg, MARGIN_NEG)

            # ---- diagonal path (exact, fp32) ----
            for mt in range(MT):
                t1 = diagpool.tile([P, D], f32, tag="t1")
                nc.sync.dma_start(out=t1, in_=z1s[mt * P:(mt + 1) * P, :])
                t2 = diagpool.tile([P, D], f32, tag="t2")
                nc.sync.dma_start(out=t2, in_=z2s[mt * P:(mt + 1) * P, :])
                dsub = diagpool.tile([P, D], f32, tag="dsub")
                # copy-then-subtract: a DVE TensorTensor has too few sync
                # wait slots to wait on two DMA queue semaphores at once
                nc.vector.tensor_copy(out=dsub, in_=t2)
                nc.vector.tensor_tensor(dsub, t1, dsub, mybir.AluOpType.subtract)
                tr1 = diagpool.tile([P, D], f32, tag="tr1")
                nc.scalar.activation(tr1, dsub, Square,
                                     accum_out=sdsum[:, mt:mt + 1])
                tr2 = diagpool.tile([P, D], f32, tag="tr2")
                nc.scalar.activation(tr2, t1, Square,
                                     accum_out=sq1sum[:, mt:mt + 1])

            nc.scalar.activation(biasP, sq1sum, Copy,
                                 bias=0.5 * MARGIN_NEG * D, scale=-0.5)
            nc.scalar.activation(stats_sb[:, MAIN_COLS:MAIN_COLS + MT], sdsum,
                                 Relu, bias=bias_mpos, scale=1.0 / D)
            nc.scalar.activation(stats_sb[:, MAIN_COLS + MT:STATS_COLS], sdsum,
                                 Relu, bias=bias_mneg, scale=-1.0 / D)

            # ---- main loop over z2 column-chunk pairs ----
            for pr in range(NPAIRS):
                pair = []
                for half in range(2):
                    ci = pr * 2 + half
                    z2c = z2pool.tile([P, KT, NCHUNK], bf16, tag="z2c")
                    nc.sync.dma_start(
                        out=z2c,
                        in_=z2T_r[:, :, ci * NCHUNK:(ci + 1) * NCHUNK])
                    # sq2 chunk row: ones^T @ (z2c*z2c), accumulated over KT
                    prow = psum_row.tile([1, NCHUNK], f32, tag="prow")
                    for k in range(KT):
                        z2sq = sqpool.tile([P, NCHUNK], bf16, tag="z2sq")
                        nc.vector.tensor_mul(z2sq, z2c[:, k, :], z2c[:, k, :])
                        nc.tensor.matmul(prow, lhsT=ones_k, rhs=z2sq,
                                         start=(k == 0), stop=(k == KT - 1))
                    msq2h = rowpool.tile([1, NCHUNK], f32, tag="msq2h")
                    nc.scalar.activation(msq2h, prow, Copy, scale=-0.5)
                    pair.append((z2c, msq2h))

                for mt in range(MT):
                    pbig = psum_main.tile([P, 2, NCHUNK], f32, tag="pbig")
                    for half in range(2):
                        z2c, msq2h = pair[half]
                        pm = pbig[:, half, :]
                        for k in range(KT):
                            nc.tensor.matmul(
                                pm,
                                lhsT=z1tT_sb[:, k, mt * P:(mt + 1) * P],
                                rhs=z2c[:, k, :],
                                start=(k == 0), stop=False)
                        nc.tensor.matmul(pm, lhsT=ones_m, rhs=msq2h,
                                         start=False, stop=True)
                    hout = scratch.tile([P, 2, NCHUNK], f32, tag="hout")
                    col = pr * MT + mt
                    nc.scalar.activation(
                        hout, pbig, Relu, bias=biasP[:, mt:mt + 1], scale=1.0,
                        accum_out=stats_sb[:, col:col + 1])

            nc.sync.dma_start(out=stats[:, :], in_=stats_sb)
    return nc


def get_nc():
    if "nc" not in _NC_CACHE:
        nc = build_nc()
        nc.finalize()
        _legalize_waits(nc)
        _NC_CACHE["nc"] = nc
    return _NC_CACHE["nc"]


def prep_in_maps(z1, z2):
    import ml_dtypes

    z1 = np.ascontiguousarray(np.asarray(z1, dtype=np.float32))
    z2 = np.ascontiguousarray(np.asarray(z2, dtype=np.float32))
    z2T = np.ascontiguousarray(z2.T.astype(ml_dtypes.bfloat16))
    in_maps = []
    for i in range(NCORES):
        sl = slice(i * M, (i + 1) * M)
        in_maps.append({
            "z1tT": np.ascontiguousarray(z1[sl].T.astype(ml_dtypes.bfloat16)),
            "z2T": z2T,
            "z1s": np.ascontiguousarray(z1[sl]),
            "z2s": np.ascontiguousarray(z2[sl]),
        })
    return in_maps


def gather(results):
    """Combine per-core stats tensors into the final scalar loss."""
    mainsum = 0.0
    lossP_sum = 0.0
    h0sum = 0.0
    for r in results:
        s = np.asarray(r["stats"], dtype=np.float64)
        mainsum += s[:, :MAIN_COLS].sum()
        lossP_sum += s[:, MAIN_COLS:MAIN_COLS + MT].sum()
        h0sum += s[:, MAIN_COLS + MT:STATS_COLS].sum()
    lossN = (mainsum * (2.0 / D) - h0sum) / (B * (B - 1))
    lossP = lossP_sum / B
    return np.array(1.5 * lossP + 0.5 * lossN, dtype=np.float32)


def run(z1, z2, trace=False, trace_cores=None):
    from concourse.bass_utils import run_bass_kernel_spmd

    nc = get_nc()
    in_maps = prep_in_maps(z1, z2)
    kwargs = {}
    if trace:
        kwargs["trace"] = True
        if trace_cores is not None:
            kwargs["trace_cores"] = trace_cores
    res = run_bass_kernel_spmd(nc, in_maps, core_ids=list(range(NCORES)),
                               **kwargs)
    return gather(res.results), res


def kernel(z1, z2):
    val, _ = run(z1, z2)
    return val


# revision 11
# speedup vs baseline: 3.1546x; 2.1711x over previous
"""Trainium2 Bass kernel: contrastive hinge loss over an 8192x8192 pairwise
mean-squared-distance matrix, sharded over 8 NeuronCores (1024 z1 rows each).

Math (reference):
  dist[m,n] = (sq1[m] + sq2[n] - 2*z1[m]@z2[n]) / D
  lossP = sum(relu(diag(dist) - MARGIN_POS)) / B
  lossN = sum(relu(MARGIN_NEG - dist) * (1-eye)) / (B*(B-1))
  out   = 1.5*lossP + 0.5*lossN

Device-side identity used for the off-diag hinge: with the PE computing
  acc[m,n] = z1[m]@z2[n] - 0.5*sq2[n]          (4 matmuls + one rank-1 fold)
we have
  relu(acc + (0.2*D - 0.5*sq1[m])) = (D/2) * relu(MARGIN_NEG - dist[m,n])
so one ScalarE activation (per-partition bias + free-axis accumulate) reduces
each PSUM tile to per-row hinge sums.  Diagonal terms (needed exactly for
lossP, and subtracted from the off-diag sum) come from a separate cheap fp32
elementwise path over the matching z1/z2 row shards.
"""

import numpy as np

B = 8192
D = 512
NCORES = 8
M = B // NCORES          # 1024 z1 rows per core
P = 128
MT = M // P              # 8 row tiles per core
NCHUNK = 512             # z2 column chunk (one PSUM bank wide)
NPAIRS = B // (2 * NCHUNK)   # 8 chunk pairs
KT = D // P              # 4 contraction subtiles
MARGIN_NEG = 0.4
MARGIN_POS = 0.01
MAIN_COLS = NPAIRS * MT          # 64 hinge-accum columns
STATS_COLS = MAIN_COLS + 2 * MT  # + lossP cols + diag-correction cols

_NC_CACHE = {}


def _legalize_waits(nc, max_waits=1):
    """This toolchain's walrus accepts only ONE sync wait command per
    instruction ("Too many sync wait commands" otherwise), while Tile
    attaches all required waits to the consuming instruction.  Hoist every
    wait beyond the first onto standalone same-engine NoOps placed directly
    before the consumer — per-engine program order keeps semantics
    identical."""
    from concourse import mybir

    for f in nc.m.functions:
        for blk in f.blocks:
            out = []
            changed = False
            for inst in blk.instructions:
                si = getattr(inst, "sync_info", None)
                waits = list(si.on_wait) if si is not None else []
                if len(waits) > max_waits:
                    for i, w in enumerate(waits[max_waits:]):
                        out.append(mybir.InstNoOp(
                            name=f"{inst.name}_hw{i}",
                            sync_info=mybir.SyncInfo(on_wait=[w], on_update=[]),
                            engine=inst.engine,
                            bass_nofuse=True,
                        ))
                    inst.sync_info = mybir.SyncInfo(
                        on_wait=waits[:max_waits], on_update=list(si.on_update))
                    changed = True
                out.append(inst)
            if changed:
                blk.instructions = out


def build_nc():
    import concourse.bass as bass
    import concourse.tile as tile
    from concourse import mybir

    f32 = mybir.dt.float32
    bf16 = mybir.dt.bfloat16
    Relu = mybir.ActivationFunctionType.Relu
    Square = mybir.ActivationFunctionType.Square
    Copy = mybir.ActivationFunctionType.Copy

    nc = bass.Bass()
    z1tT = nc.dram_tensor("z1tT", [D, M], bf16, kind="ExternalInput")
    z2T = nc.dram_tensor("z2T", [D, B], bf16, kind="ExternalInput")
    z1s = nc.dram_tensor("z1s", [M, D], f32, kind="ExternalInput")
    z2s = nc.dram_tensor("z2s", [M, D], f32, kind="ExternalInput")
    stats = nc.dram_tensor("stats", [P, STATS_COLS], f32, kind="ExternalOutput")

    z1tT_r = z1tT.rearrange("(kt p) m -> p kt m", p=P)
    z2T_r = z2T.rearrange("(kt p) n -> p kt n", p=P)

    with tile.TileContext(nc) as tc:
        with (
            tc.tile_pool(name="singles", bufs=1) as singles,
            tc.tile_pool(name="z2pool", bufs=3) as z2pool,
            tc.tile_pool(name="sqpool", bufs=4) as sqpool,
            tc.tile_pool(name="rowpool", bufs=3) as rowpool,
            tc.tile_pool(name="diagpool", bufs=3) as diagpool,
            tc.tile_pool(name="scratch", bufs=3) as scratch,
            tc.tile_pool(name="psum_row", bufs=2, space="PSUM") as psum_row,
            tc.tile_pool(name="psum_main", bufs=3, space="PSUM") as psum_main,
        ):
            # ---- resident tiles ----
            z1tT_sb = singles.tile([P, KT, M], bf16)
            nc.sync.dma_start(out=z1tT_sb, in_=z1tT_r)
            ones_k = singles.tile([P, 1], bf16)     # lhsT for partition-sums
            nc.vector.memset(ones_k, 1.0)
            ones_m = singles.tile([1, P], bf16)     # lhsT for rank-1 row fold
            nc.vector.memset(ones_m, 1.0)
            stats_sb = singles.tile([P, STATS_COLS], f32)
            sdsum = singles.tile([P, MT], f32)      # sum_d (z1-z2)^2 per row
            sq1sum = singles.tile([P, MT], f32)     # sum_d z1^2 per row
            biasP = singles.tile([P, MT], f32)      # 0.2*D - 0.5*sq1
            bias_mpos = singles.tile([P, 1], f32)
            nc.vector.memset(bias_mpos, -MARGIN_POS)
            bias_mneg = singles.tile([P, 1], f32)
            nc.vector.memset(bias_mneg, MARGIN_NEG)

            # ---- diagonal path (exact, fp32) ----
            for mt in range(MT):
                t1 = diagpool.tile([P, D], f32, tag="t1")
                nc.sync.dma_start(out=t1, in_=z1s[mt * P:(mt + 1) * P, :])
                t2 = diagpool.tile([P, D], f32, tag="t2")
                nc.sync.dma_start(out=t2, in_=z2s[mt * P:(mt + 1) * P, :])
                dsub = diagpool.tile([P, D], f32, tag="dsub")
                # copy-then-subtract: a DVE TensorTensor has too few sync
                # wait slots to wait on two DMA queue semaphores at once
                nc.vector.tensor_copy(out=dsub, in_=t2)
                nc.vector.tensor_tensor(dsub, t1, dsub, mybir.AluOpType.subtract)
                tr1 = diagpool.tile([P, D], f32, tag="tr1")
                nc.scalar.activation(tr1, dsub, Square,
                                     accum_out=sdsum[:, mt:mt + 1])
                tr2 = diagpool.tile([P, D], f32, tag="tr2")
                nc.scalar.activation(tr2, t1, Square,
                                     accum_out=sq1sum[:, mt:mt + 1])

            nc.scalar.activation(biasP, sq1sum, Copy,
                                 bias=0.5 * MARGIN_NEG * D, scale=-0.5)
            nc.scalar.activation(stats_sb[:, MAIN_COLS:MAIN_COLS + MT], sdsum,
                                 Relu, bias=bias_mpos, scale=1.0 / D)
            nc.scalar.activation(stats_sb[:, MAIN_COLS + MT:STATS_COLS], sdsum,
                                 Relu, bias=bias_mneg, scale=-1.0 / D)

            # ---- main loop over z2 column-chunk pairs ----
            for pr in range(NPAIRS):
                pair = []
                for half in range(2):
                    ci = pr * 2 + half
                    z2c = z2pool.tile([P, KT, NCHUNK], bf16, tag="z2c")
                    nc.sync.dma_start(
                        out=z2c,
                        in_=z2T_r[:, :, ci * NCHUNK:(ci + 1) * NCHUNK])
                    # sq2 chunk row: ones^T @ (z2c*z2c), accumulated over KT
                    prow = psum_row.tile([1, NCHUNK], f32, tag="prow")
                    for k in range(KT):
                        z2sq = sqpool.tile([P, NCHUNK], bf16, tag="z2sq")
                        nc.vector.tensor_mul(z2sq, z2c[:, k, :], z2c[:, k, :])
                        nc.tensor.matmul(prow, lhsT=ones_k, rhs=z2sq,
                                         start=(k == 0), stop=(k == KT - 1))
                    msq2h = rowpool.tile([1, NCHUNK], bf16, tag="msq2h")
                    nc.scalar.activation(msq2h, prow, Copy, scale=-0.5)
                    pair.append((z2c, msq2h))

                for mt in range(MT):
                    pbig = psum_main.tile([P, 2, NCHUNK], f32, tag="pbig")
                    for half in range(2):
                        z2c, msq2h = pair[half]
                        pm = pbig[:, half, :]
                        for k in range(KT):
                            nc.tensor.matmul(
                                pm,
                                lhsT=z1tT_sb[:, k, mt * P:(mt + 1) * P],
                                rhs=z2c[:, k, :],
                                start=(k == 0), stop=False)
                        nc.tensor.matmul(pm, lhsT=ones_m, rhs=msq2h,
                                         start=False, stop=True)
                    hout = scratch.tile([P, 2, NCHUNK], f32, tag="hout")
                    col = pr * MT + mt
                    nc.scalar.activation(
                        hout, pbig, Relu, bias=biasP[:, mt:mt + 1], scale=1.0,
                        accum_out=stats_sb[:, col:col + 1])

            nc.sync.dma_start(out=stats[:, :], in_=stats_sb)
    return nc


def get_nc():
    if "nc" not in _NC_CACHE:
        nc = build_nc()
        nc.finalize()
        _legalize_waits(nc)
        _NC_CACHE["nc"] = nc
    return _NC_CACHE["nc"]


def prep_in_maps(z1, z2):
    import ml_dtypes

    z1 = np.ascontiguousarray(np.asarray(z1, dtype=np.float32))
    z2 = np.ascontiguousarray(np.asarray(z2, dtype=np.float32))
    z2T = np.ascontiguousarray(z2.T.astype(ml_dtypes.bfloat16))
    in_maps = []
    for i in range(NCORES):
        sl = slice(i * M, (i + 1) * M)
        in_maps.append({
            "z1tT": np.ascontiguousarray(z1[sl].T.astype(ml_dtypes.bfloat16)),
            "z2T": z2T,
            "z1s": np.ascontiguousarray(z1[sl]),
            "z2s": np.ascontiguousarray(z2[sl]),
        })
    return in_maps


def gather(results):
    """Combine per-core stats tensors into the final scalar loss."""
    mainsum = 0.0
    lossP_sum = 0.0
    h0sum = 0.0
    for r in results:
        s = np.asarray(r["stats"], dtype=np.float64)
        mainsum += s[:, :MAIN_COLS].sum()
        lossP_sum += s[:, MAIN_COLS:MAIN_COLS + MT].sum()
        h0sum += s[:, MAIN_COLS + MT:STATS_COLS].sum()
    lossN = (mainsum * (2.0 / D) - h0sum) / (B * (B - 1))
    lossP = lossP_sum / B
    return np.array(1.5 * lossP + 0.5 * lossN, dtype=np.float32)


def run(z1, z2, trace=False, trace_cores=None):
    from concourse.bass_utils import run_bass_kernel_spmd

    nc = get_nc()
    in_maps = prep_in_maps(z1, z2)
    kwargs = {}
    if trace:
        kwargs["trace"] = True
        if trace_cores is not None:
            kwargs["trace_cores"] = trace_cores
    res = run_bass_kernel_spmd(nc, in_maps, core_ids=list(range(NCORES)),
                               **kwargs)
    return gather(res.results), res


def kernel(z1, z2):
    val, _ = run(z1, z2)
    return val


# revision 13
# speedup vs baseline: 3.3794x; 1.0713x over previous
"""Trainium2 Bass kernel: contrastive hinge loss over an 8192x8192 pairwise
mean-squared-distance matrix, sharded over 8 NeuronCores (1024 z1 rows each).

Math (reference):
  dist[m,n] = (sq1[m] + sq2[n] - 2*z1[m]@z2[n]) / D
  lossP = sum(relu(diag(dist) - MARGIN_POS)) / B
  lossN = sum(relu(MARGIN_NEG - dist) * (1-eye)) / (B*(B-1))
  out   = 1.5*lossP + 0.5*lossN

Device-side identity used for the off-diag hinge: with the PE computing
  acc[m,n] = z1[m]@z2[n] - 0.5*sq2[n]          (4 matmuls + one rank-1 fold)
we have
  relu(acc + (0.2*D - 0.5*sq1[m])) = (D/2) * relu(MARGIN_NEG - dist[m,n])
so one ScalarE activation (per-partition bias + free-axis accumulate) reduces
each PSUM tile to per-row hinge sums.  Diagonal terms (needed exactly for
lossP, and subtracted from the off-diag sum) come from a separate cheap fp32
elementwise path over the matching z1/z2 row shards.
"""

import numpy as np

B = 8192
D = 512
NCORES = 8
M = B // NCORES          # 1024 z1 rows per core
P = 128
MT = M // P              # 8 row tiles per core
NCHUNK = 512             # z2 column chunk (one PSUM bank wide)
NPAIRS = B // (2 * NCHUNK)   # 8 chunk pairs
KT = D // P              # 4 contraction subtiles
MARGIN_NEG = 0.4
MARGIN_POS = 0.01
MAIN_COLS = NPAIRS * MT          # 64 hinge-accum columns
STATS_COLS = MAIN_COLS + 2 * MT  # + lossP cols + diag-correction cols

_NC_CACHE = {}


def _legalize_waits(nc, max_waits=1):
    """This toolchain's walrus accepts only ONE sync wait command per
    instruction ("Too many sync wait commands" otherwise), while Tile
    attaches all required waits to the consuming instruction.  Hoist every
    wait beyond the first onto standalone same-engine NoOps placed directly
    before the consumer — per-engine program order keeps semantics
    identical."""
    from concourse import mybir

    for f in nc.m.functions:
        for blk in f.blocks:
            out = []
            changed = False
            for inst in blk.instructions:
                si = getattr(inst, "sync_info", None)
                waits = list(si.on_wait) if si is not None else []
                if len(waits) > max_waits:
                    for i, w in enumerate(waits[max_waits:]):
                        out.append(mybir.InstNoOp(
                            name=f"{inst.name}_hw{i}",
                            sync_info=mybir.SyncInfo(on_wait=[w], on_update=[]),
                            engine=inst.engine,
                            bass_nofuse=True,
                        ))
                    inst.sync_info = mybir.SyncInfo(
                        on_wait=waits[:max_waits], on_update=list(si.on_update))
                    changed = True
                out.append(inst)
            if changed:
                blk.instructions = out


def build_nc():
    import concourse.bass as bass
    import concourse.tile as tile
    from concourse import mybir

    f32 = mybir.dt.float32
    bf16 = mybir.dt.bfloat16
    Relu = mybir.ActivationFunctionType.Relu
    Square = mybir.ActivationFunctionType.Square
    Copy = mybir.ActivationFunctionType.Copy

    nc = bass.Bass()
    z1tT = nc.dram_tensor("z1tT", [D, M], bf16, kind="ExternalInput")
    z2T = nc.dram_tensor("z2T", [D, B], bf16, kind="ExternalInput")
    z1s = nc.dram_tensor("z1s", [M, D], f32, kind="ExternalInput")
    z2s = nc.dram_tensor("z2s", [M, D], f32, kind="ExternalInput")
    stats = nc.dram_tensor("stats", [P, STATS_COLS], f32, kind="ExternalOutput")

    z1tT_r = z1tT.rearrange("(kt p) m -> p kt m", p=P)
    z2T_r = z2T.rearrange("(kt p) n -> p kt n", p=P)

    with tile.TileContext(nc) as tc:
        with (
            tc.tile_pool(name="singles", bufs=1) as singles,
            tc.tile_pool(name="z2pool", bufs=16) as z2pool,
            tc.tile_pool(name="sqpool", bufs=6) as sqpool,
            tc.tile_pool(name="rowpool", bufs=16) as rowpool,
            tc.tile_pool(name="diagpool", bufs=3) as diagpool,
            tc.tile_pool(name="scratch", bufs=3) as scratch,
            tc.tile_pool(name="psum_row", bufs=2, space="PSUM") as psum_row,
            tc.tile_pool(name="psum_main", bufs=3, space="PSUM") as psum_main,
        ):
            # ---- resident tiles ----
            z1tT_sb = singles.tile([P, KT, M], bf16)
            nc.sync.dma_start(out=z1tT_sb, in_=z1tT_r)
            ones_k = singles.tile([P, 1], bf16)     # lhsT for partition-sums
            nc.vector.memset(ones_k, 1.0)
            ones_m = singles.tile([1, P], bf16)     # lhsT for rank-1 row fold
            nc.vector.memset(ones_m, 1.0)
            stats_sb = singles.tile([P, STATS_COLS], f32)
            sdsum = singles.tile([P, MT], f32)      # sum_d (z1-z2)^2 per row
            sq1sum = singles.tile([P, MT], f32)     # sum_d z1^2 per row
            biasP = singles.tile([P, MT], f32)      # 0.2*D - 0.5*sq1
            bias_mpos = singles.tile([P, 1], f32)
            nc.vector.memset(bias_mpos, -MARGIN_POS)
            bias_mneg = singles.tile([P, 1], f32)
            nc.vector.memset(bias_mneg, MARGIN_NEG)

            # ---- diagonal path (exact, fp32) ----
            for mt in range(MT):
                t1 = diagpool.tile([P, D], f32, tag="t1")
                nc.sync.dma_start(out=t1, in_=z1s[mt * P:(mt + 1) * P, :])
                t2 = diagpool.tile([P, D], f32, tag="t2")
                nc.sync.dma_start(out=t2, in_=z2s[mt * P:(mt + 1) * P, :])
                dsub = diagpool.tile([P, D], f32, tag="dsub")
                # copy-then-subtract: a DVE TensorTensor has too few sync
                # wait slots to wait on two DMA queue semaphores at once
                nc.vector.tensor_copy(out=dsub, in_=t2)
                nc.vector.tensor_tensor(dsub, t1, dsub, mybir.AluOpType.subtract)
                tr1 = diagpool.tile([P, D], f32, tag="tr1")
                nc.scalar.activation(tr1, dsub, Square,
                                     accum_out=sdsum[:, mt:mt + 1])
                tr2 = diagpool.tile([P, D], f32, tag="tr2")
                nc.scalar.activation(tr2, t1, Square,
                                     accum_out=sq1sum[:, mt:mt + 1])

            nc.scalar.activation(biasP, sq1sum, Copy,
                                 bias=0.5 * MARGIN_NEG * D, scale=-0.5)
            nc.scalar.activation(stats_sb[:, MAIN_COLS:MAIN_COLS + MT], sdsum,
                                 Relu, bias=bias_mpos, scale=1.0 / D)
            nc.scalar.activation(stats_sb[:, MAIN_COLS + MT:STATS_COLS], sdsum,
                                 Relu, bias=bias_mneg, scale=-1.0 / D)

            # ---- prologue: load ALL z2 chunks (8MB bf16, kept resident) and
            # compute their -0.5*sq2 rows, so the main matmul stream never
            # stalls at pair boundaries ----
            chunks = []
            for ci in range(2 * NPAIRS):
                z2c = z2pool.tile([P, KT, NCHUNK], bf16, tag="z2c")
                nc.sync.dma_start(
                    out=z2c,
                    in_=z2T_r[:, :, ci * NCHUNK:(ci + 1) * NCHUNK])
                # sq2 chunk row: ones^T @ (z2c*z2c), accumulated over KT
                prow = psum_row.tile([1, NCHUNK], f32, tag="prow")
                for k in range(KT):
                    z2sq = sqpool.tile([P, NCHUNK], bf16, tag="z2sq")
                    nc.vector.tensor_mul(z2sq, z2c[:, k, :], z2c[:, k, :])
                    nc.tensor.matmul(prow, lhsT=ones_k, rhs=z2sq,
                                     start=(k == 0), stop=(k == KT - 1))
                msq2h = rowpool.tile([1, NCHUNK], bf16, tag="msq2h")
                nc.scalar.activation(msq2h, prow, Copy, scale=-0.5)
                chunks.append((z2c, msq2h))

            # ---- main loop over z2 column-chunk pairs ----
            for pr in range(NPAIRS):
                pair = [chunks[pr * 2], chunks[pr * 2 + 1]]
                for mt in range(MT):
                    pbig = psum_main.tile([P, 2, NCHUNK], f32, tag="pbig")
                    for half in range(2):
                        z2c, msq2h = pair[half]
                        pm = pbig[:, half, :]
                        for k in range(KT):
                            nc.tensor.matmul(
                                pm,
                                lhsT=z1tT_sb[:, k, mt * P:(mt + 1) * P],
                                rhs=z2c[:, k, :],
                                start=(k == 0), stop=False)
                        nc.tensor.matmul(pm, lhsT=ones_m, rhs=msq2h,
                                         start=False, stop=True)
                    hout = scratch.tile([P, 2, NCHUNK], f32, tag="hout")
                    col = pr * MT + mt
                    nc.scalar.activation(
                        hout, pbig, Relu, bias=biasP[:, mt:mt + 1], scale=1.0,
                        accum_out=stats_sb[:, col:col + 1])

            nc.sync.dma_start(out=stats[:, :], in_=stats_sb)
    return nc


def get_nc():
    if "nc" not in _NC_CACHE:
        nc = build_nc()
        nc.finalize()
        _legalize_waits(nc)
        _NC_CACHE["nc"] = nc
    return _NC_CACHE["nc"]


def prep_in_maps(z1, z2):
    import ml_dtypes

    z1 = np.ascontiguousarray(np.asarray(z1, dtype=np.float32))
    z2 = np.ascontiguousarray(np.asarray(z2, dtype=np.float32))
    z2T = np.ascontiguousarray(z2.T.astype(ml_dtypes.bfloat16))
    in_maps = []
    for i in range(NCORES):
        sl = slice(i * M, (i + 1) * M)
        in_maps.append({
            "z1tT": np.ascontiguousarray(z1[sl].T.astype(ml_dtypes.bfloat16)),
            "z2T": z2T,
            "z1s": np.ascontiguousarray(z1[sl]),
            "z2s": np.ascontiguousarray(z2[sl]),
        })
    return in_maps


def gather(results):
    """Combine per-core stats tensors into the final scalar loss."""
    mainsum = 0.0
    lossP_sum = 0.0
    h0sum = 0.0
    for r in results:
        s = np.asarray(r["stats"], dtype=np.float64)
        mainsum += s[:, :MAIN_COLS].sum()
        lossP_sum += s[:, MAIN_COLS:MAIN_COLS + MT].sum()
        h0sum += s[:, MAIN_COLS + MT:STATS_COLS].sum()
    lossN = (mainsum * (2.0 / D) - h0sum) / (B * (B - 1))
    lossP = lossP_sum / B
    return np.array(1.5 * lossP + 0.5 * lossN, dtype=np.float32)


def run(z1, z2, trace=False, trace_cores=None):
    from concourse.bass_utils import run_bass_kernel_spmd

    nc = get_nc()
    in_maps = prep_in_maps(z1, z2)
    kwargs = {}
    if trace:
        kwargs["trace"] = True
        if trace_cores is not None:
            kwargs["trace_cores"] = trace_cores
    res = run_bass_kernel_spmd(nc, in_maps, core_ids=list(range(NCORES)),
                               **kwargs)
    return gather(res.results), res


def kernel(z1, z2):
    val, _ = run(z1, z2)
    return val
